# revision 17
# baseline (speedup 1.0000x reference)
"""Trainium2 Bass kernel for nn_AutoSelectAttention (parametric Gaussian span scores).

Computes y[b,m,k] = -(((x[k] + mean[b,m]) / (softness[b,m] + EPS))**2) + intercept[b,m]
for x[k] = k - (L-1), k in [0, 2L-1).

Sharding: the fused batch*heads dim (32) is split 4-per-core across 8 NeuronCores;
each core's [4*1024, 2047] output band is independent (no collectives).

Per-core schedule (DMA-write-roofline bound, ~33.5 MB f32 out per core):
  - host precomputes per-token planes [mean, -1/(s+eps)^2, intercept] -> one
    small input DMA; its completion (~9.6us incl. fixed preamble) gates compute.
  - x grid fp16 (exact for |int| <= 2048) built as one 512-col gpsimd iota +
    three DVE +const shifts so it's ready before the planes semaphore.
  - per block: ACT Square (z2 = (x+mean)^2, f32) then one DVE tensor_scalar
    (y = z2*ninv2 + intercept) into a grouped output tile.
  - SDMA engine hazard: a DMA with P partitions is split evenly over the
    largest engine count n <= 16 with P % n == 0 (128 -> 16 engines x 8,
    112 -> 14 engines x 8).  Engine 15 is stochastically slowed 20-50% on
    ~half the runs (shared-port contention), costing 10-20us when it carries
    a full 1/16 share.  Mixed block heights -- 18 x 128 + 16 x 112 = 4096
    tokens exactly -- cut engines 14/15 to a ~half share so they stop being
    critical even at half speed; heights interleave to spread their duty.
  - output DRAM is group-contiguous (y1a[2,128,W] singles, y2a[8,128,2W] and
    y2c[8,112,2W] pairs): each group is one contiguous DRAM region with
    8/16KB contiguous per-partition descriptors (~426 GB/s, within 1% of the
    write-side wall).  All-pairs steady state keeps readiness smooth so the
    write stream runs gap-free from ~14us at any compute cadence.
"""

import sys

import numpy as np

for _p in ("/opt/trn_rl_repo", "/root/.axon_site", "/opt/pypackages"):
    if _p not in sys.path:
        sys.path.append(_p)

L = 1024
W = 2 * L - 1  # 2047
BH = 32
M = 1024
EPS = 1e-5
NCORES = 8
BH_SH = BH // NCORES  # 4
ROWS = BH_SH * M  # 4096 tokens per core
H_A = 128  # tall blocks (all 16 SDMA engines)
H_C = 112  # short blocks (engines 0-13; engines 14/15 idle)

# Group sequence: two a-singles (early streaming), then interleaved c/a
# pairs.  18 a-blocks + 16 c-blocks = 4096 tokens exactly.
_SEQ = [("a", 1), ("a", 1)] + [("c", 2), ("a", 2)] * 8


def _make_plan():
    plan = []
    base = 0
    slots = {"y1a": 0, "y2a": 0, "y2c": 0}
    for typ, g in _SEQ:
        h = H_A if typ == "a" else H_C
        key = "y1a" if g == 1 else ("y2a" if typ == "a" else "y2c")
        bases = []
        for _ in range(g):
            bases.append(base)
            base += h
        plan.append({"key": key, "slot": slots[key], "h": h, "g": g, "bases": bases})
        slots[key] += 1
    assert base == ROWS, base
    return plan, slots


PLAN, _SLOT_COUNTS = _make_plan()
NBLK = sum(p["g"] for p in PLAN)  # 34

_NC_CACHE = {}


def _build_nc():
    import concourse.bacc as bacc
    import concourse.tile as tile
    from concourse import mybir

    f32 = mybir.dt.float32
    f16 = mybir.dt.float16
    Sq = mybir.ActivationFunctionType.Square

    nc = bacc.Bacc("TRN2", target_bir_lowering=False, debug=False)
    # planes[p, 0, k] = mean, [p, 1, k] = -1/(softness+EPS)^2, [p, 2, k] =
    # intercept for block k (compute order), token = bases[k] + p, p < h_k.
    planes = nc.dram_tensor("planes", [H_A, 3, NBLK], f32, kind="ExternalInput").ap()
    youts = {
        "y1a": nc.dram_tensor(
            "y1a", [_SLOT_COUNTS["y1a"], H_A, W], f32, kind="ExternalOutput"
        ).ap(),
        "y2a": nc.dram_tensor(
            "y2a", [_SLOT_COUNTS["y2a"], H_A, 2 * W], f32, kind="ExternalOutput"
        ).ap(),
        "y2c": nc.dram_tensor(
            "y2c", [_SLOT_COUNTS["y2c"], H_C, 2 * W], f32, kind="ExternalOutput"
        ).ap(),
    }

    with tile.TileContext(nc) as tc:
        with (
            tc.tile_pool(name="const", bufs=1) as cpool,
            tc.tile_pool(name="work", bufs=3) as wpool,
            tc.tile_pool(name="o1", bufs=2) as o1pool,
            tc.tile_pool(name="o2a", bufs=3) as o2apool,
            tc.tile_pool(name="o2c", bufs=3) as o2cpool,
        ):
            # Warmup ACTIVATE with no data dependencies: pulls the ~1.5us
            # Square table load to kernel start instead of serializing it
            # behind the planes DMA.
            warm = cpool.tile([H_A, 1], f32)
            one = nc.const_aps.tensor(1.0, (H_A, 1))
            nc.scalar.activation(warm[:], one, Sq, bias=0.0, scale=1.0)

            # x grid in fp16 (integers |x| <= 2047 are exact in fp16).
            xb = cpool.tile([H_A, 2 * L], f16)
            nc.gpsimd.iota(
                xb[:, 0:512],
                [[1, 512]],
                base=-(L - 1),
                channel_multiplier=0,
                allow_small_or_imprecise_dtypes=True,
            )
            for j in (1, 2, 3):
                nc.vector.tensor_scalar(
                    xb[:, j * 512 : (j + 1) * 512],
                    xb[:, 0:512],
                    float(j * 512),
                    None,
                    mybir.AluOpType.add,
                )

            spn = cpool.tile([H_A, 3, NBLK], f32)
            nc.sync.dma_start(spn[:], planes[:, :, :])

            pools = {"y1a": o1pool, "y2a": o2apool, "y2c": o2cpool}
            k = 0
            for grp in PLAN:
                h, g = grp["h"], grp["g"]
                ot = pools[grp["key"]].tile([h, g * W], f32)
                for j in range(g):
                    kk = k + j
                    # z2 = (x + mean)^2 on ACT (per-partition bias = mean)
                    z2 = wpool.tile([H_A, W], f32)
                    nc.scalar.activation(
                        z2[0:h, :],
                        xb[0:h, 0:W],
                        Sq,
                        bias=spn[0:h, 0, kk : kk + 1],
                        scale=1.0,
                    )
                    # y = z2 * ninv2 + intercept on DVE (per-partition scalars)
                    nc.vector.tensor_scalar(
                        ot[:, j * W : (j + 1) * W],
                        z2[0:h, :],
                        spn[0:h, 1, kk : kk + 1],
                        spn[0:h, 2, kk : kk + 1],
                        mybir.AluOpType.mult,
                        mybir.AluOpType.add,
                    )
                i = grp["slot"]
                nc.sync.dma_start(youts[grp["key"]][i : i + 1, :, :], ot[:])
                k += g
    nc.compile()
    return nc


def _get_nc():
    if "nc" not in _NC_CACHE:
        _NC_CACHE["nc"] = _build_nc()
    return _NC_CACHE["nc"]


def _make_in_maps(span: np.ndarray) -> list[dict]:
    span = np.ascontiguousarray(span, dtype=np.float32)
    in_maps = []
    for c in range(NCORES):
        flat = span[c * BH_SH : (c + 1) * BH_SH].reshape(ROWS, 3)
        planes = np.zeros((H_A, 3, NBLK), np.float32)
        planes[:, 1, :] = -1.0  # harmless pad for rows >= h in short blocks
        k = 0
        for grp in PLAN:
            h = grp["h"]
            for j in range(grp["g"]):
                tok = flat[grp["bases"][j] : grp["bases"][j] + h]
                planes[:h, 0, k] = tok[:, 0]
                planes[:h, 1, k] = (
                    -1.0 / (tok[:, 1].astype(np.float64) + EPS) ** 2
                ).astype(np.float32)
                planes[:h, 2, k] = tok[:, 2]
                k += 1
        in_maps.append({"planes": np.ascontiguousarray(planes)})
    return in_maps


def kernel(span: np.ndarray, _trace: bool = False, _tmpdir: str | None = None):
    from concourse.bass_utils import run_bass_kernel_spmd

    span = np.ascontiguousarray(span, dtype=np.float32)
    nc = _get_nc()
    in_maps = _make_in_maps(span)
    res = run_bass_kernel_spmd(
        nc,
        in_maps,
        core_ids=list(range(NCORES)),
        trace=_trace,
        tmpdir=_tmpdir,
    )
    # Reassemble each core's [ROWS, W] band: group slot i holds [h, g, W]
    # with token t = bases[j] + p.
    shards = []
    for r in res.results:
        band = np.empty((ROWS, W), np.float32)
        for grp in PLAN:
            h, g, i = grp["h"], grp["g"], grp["slot"]
            arr = np.asarray(r[grp["key"]]).reshape(-1, h, g * W)[i]
            blocks = arr.reshape(h, g, W).transpose(1, 0, 2)  # [g, h, W]
            for j in range(g):
                b0 = grp["bases"][j]
                band[b0 : b0 + h, :] = blocks[j]
        shards.append(band.reshape(BH_SH, M, W))
    out = np.concatenate(shards, axis=0).astype(np.float32)
    if _trace:
        kernel.last_results = res
    return out


# revision 18
# speedup vs baseline: 1.1187x; 1.1187x over previous
"""Trainium2 Bass kernel for nn_AutoSelectAttention (parametric Gaussian span scores).

Computes y[b,m,k] = -(((x[k] + mean[b,m]) / (softness[b,m] + EPS))**2) + intercept[b,m]
for x[k] = k - (L-1), k in [0, 2L-1).

Sharding: the fused batch*heads dim (32) is split 4-per-core across 8 NeuronCores;
each core's [4*1024, 2047] output band is independent (no collectives).

Per-core schedule (DMA-write-roofline bound, ~33.5 MB f32 out per core):
  - host precomputes per-token planes [mean, -1/(s+eps)^2, intercept] -> one
    small input DMA; its completion (~9.6us incl. fixed preamble) gates compute.
  - x grid [128, 2047] fp16 (exact for |int| <= 2048) built as one 512-col
    gpsimd iota + three DVE +const shifts so it's ready before the planes
    semaphore (a full-width iota would block the first block until ~11.9us).
  - per 128-token block: ACT Square (z2 = (x+mean)^2, f32) then one DVE
    tensor_scalar (y = z2*ninv2 + intercept) into a grouped output tile.
  - output DRAM is group-contiguous (y1[2,128,W] singles, y2[15,128,2W]
    pairs): each group is one fully contiguous 1-2MB DRAM region whose
    per-partition descriptor is 8188/16376 contiguous bytes (16KB
    descriptors run at ~426 GB/s, within 1% of the per-core write-side
    wall).  Two singles start the write stream at ~14us; all-pairs steady
    state keeps DMA readiness smooth (pair ready-cadence < pair transfer
    time at any observed compute cadence), so the stream runs gap-free.
  - all DMAs keep the full 128 partitions: the descriptor generator splits
    P partitions over the largest n <= 16 with P % n == 0, and any config
    mixing in 112/120/124-partition DMAs measured 25-60% slower chip-wide
    under 8-core HBM contention (tested and rejected).
"""

import sys

import numpy as np

for _p in ("/opt/trn_rl_repo", "/root/.axon_site", "/opt/pypackages"):
    if _p not in sys.path:
        sys.path.append(_p)

L = 1024
W = 2 * L - 1  # 2047
BH = 32
M = 1024
EPS = 1e-5
NCORES = 8
BH_SH = BH // NCORES  # 4
ROWS = BH_SH * M  # 4096 tokens per core
P = 128
NBLK = ROWS // P  # 32 blocks of 128 tokens

# Output DMA grouping (must sum to NBLK): two singles so the write stream
# starts as early as possible, then pairs for 16KB descriptors with smooth
# readiness.
GROUPS = [1, 1] + [2] * 15
assert sum(GROUPS) == NBLK

_NC_CACHE = {}


def _build_nc():
    import concourse.bacc as bacc
    import concourse.tile as tile
    from concourse import mybir

    f32 = mybir.dt.float32
    f16 = mybir.dt.float16
    Sq = mybir.ActivationFunctionType.Square

    nc = bacc.Bacc("TRN2", target_bir_lowering=False, debug=False)
    # planes[p, 0, k] = mean, [p, 1, k] = -1/(softness+EPS)^2, [p, 2, k] =
    # intercept for token t = k*128 + p (host-precomputed).
    planes = nc.dram_tensor("planes", [P, 3, NBLK], f32, kind="ExternalInput").ap()
    # One output tensor per group size; group i of size g occupies one fully
    # contiguous g*1MB DRAM region laid out [partition, g*W] so every
    # partition's descriptor is g*8188 contiguous bytes and partitions are
    # adjacent (y*[i, p, j*W+w] = out[token (k0+j)*128+p, w]).
    n_by_g = {g: GROUPS.count(g) for g in set(GROUPS)}
    youts = {
        g: nc.dram_tensor(f"y{g}", [n, P, g * W], f32, kind="ExternalOutput").ap()
        for g, n in sorted(n_by_g.items())
    }

    with tile.TileContext(nc) as tc:
        with (
            tc.tile_pool(name="const", bufs=1) as cpool,
            tc.tile_pool(name="work", bufs=3) as wpool,
            tc.tile_pool(name="o1", bufs=2) as o1pool,
            tc.tile_pool(name="o2", bufs=5) as o2pool,
        ):
            # Warmup ACTIVATE with no data dependencies: pulls the ~1.5us
            # Square table load to kernel start instead of serializing it
            # behind the planes DMA.
            warm = cpool.tile([P, 1], f32)
            one = nc.const_aps.tensor(1.0, (P, 1))
            nc.scalar.activation(warm[:], one, Sq, bias=0.0, scale=1.0)

            # x grid in fp16 (integers |x| <= 2047 are exact in fp16).
            xb = cpool.tile([P, 2 * L], f16)
            nc.gpsimd.iota(
                xb[:, 0:512],
                [[1, 512]],
                base=-(L - 1),
                channel_multiplier=0,
                allow_small_or_imprecise_dtypes=True,
            )
            for j in (1, 2, 3):
                nc.vector.tensor_scalar(
                    xb[:, j * 512 : (j + 1) * 512],
                    xb[:, 0:512],
                    float(j * 512),
                    None,
                    mybir.AluOpType.add,
                )

            spn = cpool.tile([P, 3, NBLK], f32)
            nc.sync.dma_start(spn[:], planes[:, :, :])

            pools = {1: o1pool, 2: o2pool}
            gidx = {g: 0 for g in n_by_g}
            k = 0
            for g in GROUPS:
                ot = pools[g].tile([P, g * W], f32)
                for j in range(g):
                    kk = k + j
                    # z2 = (x + mean)^2 on ACT (per-partition bias = mean)
                    z2 = wpool.tile([P, W], f32)
                    nc.scalar.activation(
                        z2[:], xb[:, 0:W], Sq, bias=spn[:, 0, kk : kk + 1], scale=1.0
                    )
                    # y = z2 * ninv2 + intercept on DVE (per-partition scalars)
                    nc.vector.tensor_scalar(
                        ot[:, j * W : (j + 1) * W],
                        z2[:],
                        spn[:, 1, kk : kk + 1],
                        spn[:, 2, kk : kk + 1],
                        mybir.AluOpType.mult,
                        mybir.AluOpType.add,
                    )
                i = gidx[g]
                nc.sync.dma_start(youts[g][i : i + 1, :, :], ot[:])
                gidx[g] += 1
                k += g
    nc.compile()
    return nc


def _get_nc():
    if "nc" not in _NC_CACHE:
        _NC_CACHE["nc"] = _build_nc()
    return _NC_CACHE["nc"]


def _make_in_maps(span: np.ndarray) -> list[dict]:
    span = np.ascontiguousarray(span, dtype=np.float32)
    in_maps = []
    for c in range(NCORES):
        # [blk, p, comp] with token t = blk*128 + p
        shard = span[c * BH_SH : (c + 1) * BH_SH].reshape(NBLK, P, 3)
        mean = shard[:, :, 0].T  # [p, blk]
        soft = shard[:, :, 1].T.astype(np.float64)
        cept = shard[:, :, 2].T
        ninv2 = (-1.0 / (soft + EPS) ** 2).astype(np.float32)
        planes = np.ascontiguousarray(
            np.stack([mean, ninv2, cept], axis=1), dtype=np.float32
        )  # [128, 3, NBLK]
        in_maps.append({"planes": planes})
    return in_maps


def kernel(span: np.ndarray, _trace: bool = False, _tmpdir: str | None = None):
    from concourse.bass_utils import run_bass_kernel_spmd

    nc = _get_nc()
    in_maps = _make_in_maps(span)
    res = run_bass_kernel_spmd(
        nc,
        in_maps,
        core_ids=list(range(NCORES)),
        trace=_trace,
        tmpdir=_tmpdir,
    )
    # Reassemble each core's [ROWS, W] band from the group-contiguous
    # tensors: group i of size g holds [P, g, W] with token t = (k0+j)*128+p.
    shards = []
    for r in res.results:
        band = np.empty((ROWS, W), np.float32)
        gidx = {g: 0 for g in set(GROUPS)}
        k = 0
        for g in GROUPS:
            i = gidx[g]
            arr = np.asarray(r[f"y{g}"]).reshape(-1, P, g * W)[i]
            band[k * P : (k + g) * P, :] = (
                arr.reshape(P, g, W).transpose(1, 0, 2).reshape(g * P, W)
            )
            gidx[g] += 1
            k += g
        shards.append(band.reshape(BH_SH, M, W))
    out = np.concatenate(shards, axis=0).astype(np.float32)
    if _trace:
        kernel.last_results = res
    return out
